# revision 10
# baseline (speedup 1.0000x reference)
"""DGM-net forward kernel for Trainium2, 8-core data parallel.

Network (per batch row x of width 101, n_nodes=512, 3 layers):
    S = tanh(x @ W0 + b0)
    for i in 0..2:
        Z = tanh(x @ Uz[i] + S @ Wz[i] + bz[i])
        G = tanh(x @ Ug[i] + S @ Wg[i] + bg[i])
        R = tanh(x @ Ur[i] + S @ Wr[i] + br[i])
        H = tanh(x @ Uh[i] + (S*R) @ Wh[i] + bh[i])
        S = (1-G)*H + Z*S
    out = S @ Wf + bf

Layout: activations feature-major ([feature partitions, batch free]) so
every matmul uses the weight in NATURAL layout as stationary lhsT and
the activation as moving rhs.  X is host-transposed and shipped as fp32r
"XT" with a ones row at partition 0; every bias is folded into the
matmul (U/W0 stationaries carry the bias as row 0), so ACT instructions
are bias-free and can span two PSUM banks.

HW-microbenchmarked facts this design is built on (mb.py):
  fp32r MM [128x128]x[128x512]    ~152 ns    (bf16 is SLOWER: 190 ns)
  ACT tanh [128,512] PSUM->SBUF   ~458 ns
  DVE op [128,512]                ~290-320 ns
  cross-engine dependency edge    ~1.1 us (!!)  -- 10x the cost model
The ~1.1us/edge handoff latency makes serial MM->ACT->DVE->MM chains
brutally expensive, so v4 INTERLEAVES TWO CHUNKS' instruction streams
unit-by-unit: every dependency stall of chunk A is covered by ~1.5us of
independent PE work from chunk B.  PSUM accumulation groups span two
banks ([128,2,512], one bias-free ACT each); DVE runs at half-gate
width; Z/G/H gate tiles are bf16 (numpy-validated rel err 3.7e-3 vs
2e-2 budget) so two chunks' tiles fit in SBUF.

Matmuls run as float32r (1 cycle/row).  fp32r operands must be produced
as fp32r, so weight DRAM params are declared float32r (DMA passthrough)
and the tanh/DVE producers of matmul operands write fp32r directly.
"""
import numpy as np
from contextlib import ExitStack

import concourse.bacc as bacc
import concourse.mybir as mybir
import concourse.tile as tile
from concourse.bass_utils import run_bass_kernel_spmd


N_CORES = 8
B_FULL = 65536
B = B_FULL // N_CORES      # rows per core
D = 101                    # input width
DA = D + 1                 # augmented with ones row (bias fold)
N = 512                    # n_nodes
L = 3                      # layers
BT = 512                   # batch chunk (free dim of matmuls)
NT = N // 128              # output-feature tiles per gate
KT = N // 128              # contraction tiles for S@W
NP = NT // 2               # two-bank pair groups per gate
FP = mybir.dt.float32
FR = mybir.dt.float32r
BF = mybir.dt.bfloat16

GATES = ("z", "g", "r", "h")


def _build(reps=1):
    nc = bacc.Bacc(None)
    Tanh = mybir.ActivationFunctionType.Tanh

    XTd = nc.declare_dram_parameter("XT", [DA, B], FR, isOutput=False)
    W0d = nc.declare_dram_parameter("W0a", [DA, N], FR, isOutput=False)
    Ud = {g: nc.declare_dram_parameter(f"U{g}a", [L, DA, N], FR, isOutput=False)
          for g in GATES}
    Wd = {g: nc.declare_dram_parameter(f"W{g}", [L, N, N], FR, isOutput=False)
          for g in GATES}
    Wfd = nc.declare_dram_parameter("Wf", [N, 1], FR, isOutput=False)
    bfd = nc.declare_dram_parameter("bfc", [1, 1], FR, isOutput=False)
    OUT = nc.declare_dram_parameter("out", [B, 1], FP, isOutput=True)

    with tile.TileContext(nc) as tc, ExitStack() as ctx:
        consts = ctx.enter_context(tc.tile_pool(name="consts", bufs=1))
        xtpool = ctx.enter_context(tc.tile_pool(name="xt", bufs=4))
        spool = ctx.enter_context(tc.tile_pool(name="s", bufs=2))
        zpool = ctx.enter_context(tc.tile_pool(name="z", bufs=2))
        gpool = ctx.enter_context(tc.tile_pool(name="g", bufs=2))
        rpool = ctx.enter_context(tc.tile_pool(name="r", bufs=2))
        hpool = ctx.enter_context(tc.tile_pool(name="h", bufs=2))
        opool = ctx.enter_context(tc.tile_pool(name="o", bufs=2))
        # pair-granular PSUM: each tile spans TWO banks ([128, 2, 512] fp32)
        psum = ctx.enter_context(tc.tile_pool(name="psum", bufs=3, space="PSUM"))
        psum_f = ctx.enter_context(tc.tile_pool(name="psum_f", bufs=2, space="PSUM"))

        # --- resident weights, natural (k-major) layout, via SWDGE ---
        def wdma(out, in_):
            nc.gpsimd.dma_start(out=out, in_=in_)

        w0 = consts.tile([DA, N], FR)
        bfc = consts.tile([1, 1], FR)
        u0, w0g, u12, w12 = {}, {}, {}, {}
        for g in GATES:
            u0[g] = consts.tile([DA, N], FR, name=f"u0_{g}")
            w0g[g] = consts.tile([128, KT, N], FR, name=f"w0_{g}")
            u12[g] = consts.tile([DA, L - 1, N], FR, name=f"u12_{g}")
            w12[g] = consts.tile([128, L - 1, KT, N], FR, name=f"w12_{g}")
        wf = consts.tile([128, KT], FR)

        def u_ap(g, l, c0, c1):
            return u0[g][:, c0:c1] if l == 0 else u12[g][:, l - 1, c0:c1]

        def w_ap(g, l, kt, c0, c1):
            return (w0g[g][:, kt, c0:c1] if l == 0
                    else w12[g][:, l - 1, kt, c0:c1])

        def emit_weight_dmas():
            nc.sync.dma_start(out=bfc[:], in_=bfd[:])
            wdma(w0[:], W0d[:])
            # first-consumed layer-0 weights first (gate order r,z,g,h)
            for g in ("r", "z", "g", "h"):
                wdma(u0[g][:], Ud[g][0].rearrange("p n -> p n"))
                wdma(w0g[g][:, 0:2],
                     Wd[g][0, 0:256].rearrange("(kt p) n -> p kt n", p=128))
                wdma(w0g[g][:, 2:4],
                     Wd[g][0, 256:512].rearrange("(kt p) n -> p kt n", p=128))
            for g in ("r", "z", "g", "h"):
                wdma(u12[g][:], Ud[g][1:3].rearrange("l p n -> p l n"))
                wdma(w12[g][:, 0],
                     Wd[g][1].rearrange("(kt p) n -> p kt n", p=128))
            for g in ("r", "z", "g", "h"):
                wdma(w12[g][:, 1],
                     Wd[g][2].rearrange("(kt p) n -> p kt n", p=128))
            wdma(wf[:], Wfd[:].rearrange("(kt p) o -> p (kt o)", p=128))

        sub, mult = mybir.AluOpType.subtract, mybir.AluOpType.mult

        def f32(ap):            # read fp32r bits as plain fp32 (DVE/ACT reads)
            return ap.bitcast(FP) if ap.dtype == FR else ap

        def load_xt(c):
            xt = xtpool.tile([DA, BT], FR)
            if c == 0:
                h = BT // 2
                nc.sync.dma_start(out=xt[:, 0:h], in_=XTd[:, 0:h])
                nc.sync.dma_start(out=xt[:, h:BT], in_=XTd[:, h:BT])
            else:
                eng = nc.scalar if c == 1 else nc.sync
                eng.dma_start(out=xt[:], in_=XTd[:, c * BT:(c + 1) * BT])
            return xt

        def emit_s0_pair(xt, s, np_):
            # S0 = tanh(X_aug @ W0_aug), one two-bank pair group
            acc = psum.tile([128, 2, BT], FP)
            for i in range(2):
                nt = 2 * np_ + i
                nc.tensor.matmul(acc[:, i, :], w0[:, nt * 128:(nt + 1) * 128],
                                 xt[:], start=True, stop=True)
            nc.scalar.activation(s[:, 2 * np_:2 * np_ + 2, :], acc[:], Tanh)

        def emit_gate_pair(g, l, xt, src, np_, dest):
            # one PSUM pair-group: the two S-independent X@U matmuls FIRST
            # (free PE work while dependency chains drain), then S@W
            # kt-MAJOR across both bank slices so the first-half S update
            # unlocks 4 of 8 matmuls early.
            acc = psum.tile([128, 2, BT], FP)
            for i in range(2):
                nt = 2 * np_ + i
                nc.tensor.matmul(
                    acc[:, i, :], u_ap(g, l, nt * 128, (nt + 1) * 128),
                    xt[:], start=True, stop=False)
            for kt in range(KT):
                for i in range(2):
                    nt = 2 * np_ + i
                    nc.tensor.matmul(
                        acc[:, i, :],
                        w_ap(g, l, kt, nt * 128, (nt + 1) * 128),
                        src[:, kt, :], start=False, stop=(kt == KT - 1))
            nc.scalar.activation(dest[:, 2 * np_:2 * np_ + 2, :], acc[:], Tanh)

        def chunk_units(c, xt, s):
            """Generator: one yield per work unit.  Two of these run zipped
            so units of the paired chunk interleave 1:1."""
            for l in range(L):
                rt = rpool.tile([128, NT, BT], FR)
                zt = zpool.tile([128, NT, BT], BF)
                gt = gpool.tile([128, NT, BT], BF)
                ht = hpool.tile([128, NT, BT], BF)
                # R first: hides the R-ACT -> R-mul -> H chain under Z/G
                for np_ in range(NP):
                    emit_gate_pair("r", l, xt, s, np_, rt)
                    yield
                for np_ in range(NP):
                    emit_gate_pair("z", l, xt, s, np_, zt)
                    yield
                # DVE batch 1: R <- S*R (feeds H), Z <- Z*S (frees Z-ACT dep)
                for hf in range(2):
                    cs = slice(2 * hf, 2 * hf + 2)
                    nc.vector.tensor_mul(rt[:, cs, :], f32(s[:, cs, :]),
                                         f32(rt[:, cs, :]))
                for hf in range(2):
                    cs = slice(2 * hf, 2 * hf + 2)
                    nc.vector.tensor_mul(zt[:, cs, :], zt[:, cs, :],
                                         f32(s[:, cs, :]))
                yield
                for np_ in range(NP):
                    emit_gate_pair("g", l, xt, s, np_, gt)
                    yield
                for np_ in range(NP):
                    emit_gate_pair("h", l, xt, rt, np_, ht)
                    yield
                # DVE batch 2: S = (Z*S) - (G-1)*H, half-gate granular
                for hf in range(2):
                    cs = slice(2 * hf, 2 * hf + 2)
                    nc.vector.scalar_tensor_tensor(
                        gt[:, cs, :], gt[:, cs, :], 1.0, ht[:, cs, :],
                        op0=sub, op1=mult)          # (G-1)*H
                    nc.vector.tensor_sub(s[:, cs, :], zt[:, cs, :],
                                         gt[:, cs, :])
                yield
            # final: out = S @ Wf + bf (bf via K=1 matmul on the ones row)
            accf = psum_f.tile([1, BT], FP)
            nc.tensor.matmul(accf[:], bfc[:], xt[0:1, :],
                             start=True, stop=False)
            for kt in range(KT):
                nc.tensor.matmul(accf[:], wf[:, kt:kt + 1], s[:, kt, :],
                                 start=False, stop=(kt == KT - 1))
            ot = opool.tile([1, BT], FP)
            nc.scalar.activation(ot[:], accf[:],
                                 mybir.ActivationFunctionType.Copy)
            r0 = c * BT
            nc.sync.dma_start(out=OUT[r0:r0 + BT, 0:1].rearrange("b o -> o b"),
                              in_=ot[:])
            yield

        def emit_all():
            n_chunks = B // BT
            xts = {0: load_xt(0), 1: load_xt(1)}
            # startup S0 for the first pair (batch-halved chunk 0 so the PE
            # starts on the first xt half-transfer)
            s_a = spool.tile([128, KT, BT], FR, name="s")
            for h in range(2):
                c0, c1 = h * 256, (h + 1) * 256
                for np_ in range(NP):
                    acc = psum.tile([128, 2, BT], FP, name="acc")
                    for i in range(2):
                        nt = 2 * np_ + i
                        nc.tensor.matmul(acc[:, i, 0:256],
                                         w0[:, nt * 128:(nt + 1) * 128],
                                         xts[0][:, c0:c1], start=True,
                                         stop=True)
                    nc.scalar.activation(s_a[:, 2 * np_:2 * np_ + 2, c0:c1],
                                         acc[:, :, 0:256], Tanh)
            s_b = spool.tile([128, KT, BT], FR, name="s")
            for np_ in range(NP):
                emit_s0_pair(xts[1], s_b, np_)
            for p in range(n_chunks // 2):
                ca, cb = 2 * p, 2 * p + 1
                if ca + 2 < n_chunks:
                    xts[ca + 2] = load_xt(ca + 2)
                if cb + 2 < n_chunks:
                    xts[cb + 2] = load_xt(cb + 2)
                for _ in zip(chunk_units(ca, xts[ca], s_a),
                             chunk_units(cb, xts[cb], s_b)):
                    pass
                if ca + 2 < n_chunks:
                    # next pair's S0 (covers this pair's final/update tails;
                    # spool bufs=2 rotation lands these in the pair's freed
                    # buffers, WAR-released by the finals just emitted)
                    s_a2 = spool.tile([128, KT, BT], FR, name="s")
                    for np_ in range(NP):
                        emit_s0_pair(xts[ca + 2], s_a2, np_)
                    s_b2 = spool.tile([128, KT, BT], FR, name="s")
                    for np_ in range(NP):
                        emit_s0_pair(xts[cb + 2], s_b2, np_)
                    s_a, s_b = s_a2, s_b2

        emit_weight_dmas()
        if reps == 1:
            emit_all()
        else:           # device-side repetition loop, for benchmarking only
            with tc.For_i(0, reps):
                emit_all()

    nc.compile()
    return nc


_NC = None


def _get_nc():
    global _NC
    if _NC is None:
        _NC = _build()
    return _NC


def prep_shared(inputs):
    """Augment U-type weights with their bias as ROW 0 (matches the ones
    row at partition 0 of XT); pass W/Wf through."""
    shared = {}
    for g in GATES:
        shared[f"W{g}"] = np.ascontiguousarray(
            np.asarray(inputs[f"W{g}"], np.float32))
        U = np.asarray(inputs[f"U{g}"], np.float32)          # [L, D, N]
        b = np.asarray(inputs[f"b{g}"], np.float32)          # [L, 1, N]
        shared[f"U{g}a"] = np.ascontiguousarray(
            np.concatenate([b.reshape(L, 1, N), U], axis=1))  # [L, DA, N]
    W0 = np.asarray(inputs["W0"], np.float32)                # [D, N]
    b0 = np.asarray(inputs["b0"], np.float32)                # [1, N]
    shared["W0a"] = np.ascontiguousarray(np.concatenate([b0, W0], axis=0))
    shared["Wf"] = np.ascontiguousarray(np.asarray(inputs["Wf"], np.float32))
    shared["bfc"] = np.asarray(inputs["bf"], np.float32).reshape(1, 1)
    return shared


def prep_xt(Xcore):
    """[B, D] batch-major core shard -> [DA, B] feature-major, ones row 0."""
    Xt = np.asarray(Xcore, np.float32).T                     # [D, B]
    ones = np.ones((1, Xt.shape[1]), np.float32)
    return np.ascontiguousarray(np.concatenate([ones, Xt], axis=0))


def _run(inputs, **kw):
    nc = _get_nc()
    shared = prep_shared(inputs)
    X = np.asarray(inputs["X"], np.float32)
    in_maps = [dict(shared, XT=prep_xt(X[i * B:(i + 1) * B]))
               for i in range(N_CORES)]
    res = run_bass_kernel_spmd(nc, in_maps, list(range(N_CORES)), **kw)
    out = np.concatenate([res.results[i]["out"] for i in range(N_CORES)], axis=0)
    return out, res


def kernel(**inputs) -> np.ndarray:
    out, _ = _run(inputs)
    return out


# revision 14
# speedup vs baseline: 1.0796x; 1.0796x over previous
"""DGM-net forward kernel for Trainium2, 8-core data parallel.

Network (per batch row x of width 101, n_nodes=512, 3 layers):
    S = tanh(x @ W0 + b0)
    for i in 0..2:
        Z = tanh(x @ Uz[i] + S @ Wz[i] + bz[i])
        G = tanh(x @ Ug[i] + S @ Wg[i] + bg[i])
        R = tanh(x @ Ur[i] + S @ Wr[i] + br[i])
        H = tanh(x @ Uh[i] + (S*R) @ Wh[i] + bh[i])
        S = (1-G)*H + Z*S
    out = S @ Wf + bf

Layout: activations feature-major ([feature partitions, batch free]) so
every matmul uses the weight in NATURAL layout as stationary lhsT and
the activation as moving rhs.  X is host-transposed and shipped as bf16
"XT" with a ones row at partition 0; every bias is folded into the
matmul (U/W0 stationaries carry the bias as row 0), so ACT instructions
are bias-free and can span two PSUM banks.

HW-microbenchmarked facts (mb.py, REAL random data -- the zero-data
regime is ~1.5x faster and misleading):
  bf16  MM [128x128]x[128x512]    ~199 ns   <- fastest real-data option
  fp32r MM                        ~220 ns
  ACT tanh pair [128,2x512]       ~690 ns
  DVE op [128,512]                ~370 ns
  cross-engine dependency edge    ~1.1 us   -- 10x the cost model
The whole compute path is bf16 (numpy-validated rel err 7e-3 vs 2e-2
budget; PSUM accumulation stays fp32): 10% faster matmul stream AND
half the SBUF/DMA.  The ~1.1us/edge handoff makes serial MM->ACT->DVE->
MM chains expensive, so every layer boundary is covered with independent
PE work: the previous chunk's deferred final, chunk c+2's S0, and the
next layer's hoisted X@U matmuls; within a group the S@W contraction is
kt-major so the first-half S update unlocks half the matmuls early.
"""
import numpy as np
import ml_dtypes
from contextlib import ExitStack

import concourse.bacc as bacc
import concourse.mybir as mybir
import concourse.tile as tile
from concourse.bass_utils import run_bass_kernel_spmd


N_CORES = 8
B_FULL = 65536
B = B_FULL // N_CORES      # rows per core
D = 101                    # input width
DA = D + 1                 # augmented with ones row (bias fold)
N = 512                    # n_nodes
L = 3                      # layers
BT = 512                   # batch chunk (free dim of matmuls)
NT = N // 128              # output-feature tiles per gate
KT = N // 128              # contraction tiles for S@W
NP = NT // 2               # two-bank pair groups per gate
FP = mybir.dt.float32
BF = mybir.dt.bfloat16

GATES = ("z", "g", "r", "h")


def _build(reps=1):
    nc = bacc.Bacc(None)
    Tanh = mybir.ActivationFunctionType.Tanh

    XTd = nc.declare_dram_parameter("XT", [DA, B], BF, isOutput=False)
    W0d = nc.declare_dram_parameter("W0a", [DA, N], BF, isOutput=False)
    Ud = {g: nc.declare_dram_parameter(f"U{g}a", [L, DA, N], BF, isOutput=False)
          for g in GATES}
    Wd = {g: nc.declare_dram_parameter(f"W{g}", [L, N, N], BF, isOutput=False)
          for g in GATES}
    Wfd = nc.declare_dram_parameter("Wf", [N, 1], BF, isOutput=False)
    bfd = nc.declare_dram_parameter("bfc", [1, 1], BF, isOutput=False)
    OUT = nc.declare_dram_parameter("out", [B, 1], FP, isOutput=True)

    with tile.TileContext(nc) as tc, ExitStack() as ctx:
        consts = ctx.enter_context(tc.tile_pool(name="consts", bufs=1))
        xtpool = ctx.enter_context(tc.tile_pool(name="xt", bufs=4))
        spool = ctx.enter_context(tc.tile_pool(name="s", bufs=3))
        zpool = ctx.enter_context(tc.tile_pool(name="z", bufs=1))
        gpool = ctx.enter_context(tc.tile_pool(name="g", bufs=1))
        rpool = ctx.enter_context(tc.tile_pool(name="r", bufs=1))
        hpool = ctx.enter_context(tc.tile_pool(name="h", bufs=1))
        opool = ctx.enter_context(tc.tile_pool(name="o", bufs=2))
        # pair-granular PSUM: each tile spans TWO banks ([128, 2, 512] fp32)
        psum = ctx.enter_context(tc.tile_pool(name="psum", bufs=3, space="PSUM"))
        psum_f = ctx.enter_context(tc.tile_pool(name="psum_f", bufs=2, space="PSUM"))

        # --- resident weights, natural (k-major) layout, via SWDGE ---
        def wdma(out, in_):
            nc.gpsimd.dma_start(out=out, in_=in_)

        w0 = consts.tile([DA, N], BF)
        bfc = consts.tile([1, 1], BF)
        u0, w0g, u12, w12 = {}, {}, {}, {}
        for g in GATES:
            u0[g] = consts.tile([DA, N], BF, name=f"u0_{g}")
            w0g[g] = consts.tile([128, KT, N], BF, name=f"w0_{g}")
            u12[g] = consts.tile([DA, L - 1, N], BF, name=f"u12_{g}")
            w12[g] = consts.tile([128, L - 1, KT, N], BF, name=f"w12_{g}")
        wf = consts.tile([128, KT], BF)

        def u_ap(g, l, c0, c1):
            return u0[g][:, c0:c1] if l == 0 else u12[g][:, l - 1, c0:c1]

        def w_ap(g, l, kt, c0, c1):
            return (w0g[g][:, kt, c0:c1] if l == 0
                    else w12[g][:, l - 1, kt, c0:c1])

        def emit_weight_dmas():
            nc.sync.dma_start(out=bfc[:], in_=bfd[:])
            wdma(w0[:], W0d[:])
            # first-consumed layer-0 weights first (gate order r,z,g,h)
            for g in ("r", "z", "g", "h"):
                wdma(u0[g][:], Ud[g][0].rearrange("p n -> p n"))
                wdma(w0g[g][:, 0:2],
                     Wd[g][0, 0:256].rearrange("(kt p) n -> p kt n", p=128))
                wdma(w0g[g][:, 2:4],
                     Wd[g][0, 256:512].rearrange("(kt p) n -> p kt n", p=128))
            for g in ("r", "z", "g", "h"):
                wdma(u12[g][:], Ud[g][1:3].rearrange("l p n -> p l n"))
                wdma(w12[g][:, 0],
                     Wd[g][1].rearrange("(kt p) n -> p kt n", p=128))
            for g in ("r", "z", "g", "h"):
                wdma(w12[g][:, 1],
                     Wd[g][2].rearrange("(kt p) n -> p kt n", p=128))
            wdma(wf[:], Wfd[:].rearrange("(kt p) o -> p (kt o)", p=128))

        sub, mult = mybir.AluOpType.subtract, mybir.AluOpType.mult

        def load_xt(c):
            xt = xtpool.tile([DA, BT], BF)
            if c == 0:
                h = BT // 2
                nc.sync.dma_start(out=xt[:, 0:h], in_=XTd[:, 0:h])
                nc.sync.dma_start(out=xt[:, h:BT], in_=XTd[:, h:BT])
            else:
                eng = nc.scalar if c == 1 else nc.sync
                eng.dma_start(out=xt[:], in_=XTd[:, c * BT:(c + 1) * BT])
            return xt

        def emit_s0_pair(xt, s, np_):
            # S0 = tanh(X_aug @ W0_aug), one two-bank pair group
            acc = psum.tile([128, 2, BT], FP, name="acc")
            for i in range(2):
                nt = 2 * np_ + i
                nc.tensor.matmul(acc[:, i, :], w0[:, nt * 128:(nt + 1) * 128],
                                 xt[:], start=True, stop=True)
            nc.scalar.activation(s[:, 2 * np_:2 * np_ + 2, :], acc[:], Tanh)

        def emit_xu(acc, g, l, xt, np_):
            # the two S-independent X@U matmuls of a pair group
            for i in range(2):
                nt = 2 * np_ + i
                nc.tensor.matmul(
                    acc[:, i, :], u_ap(g, l, nt * 128, (nt + 1) * 128),
                    xt[:], start=True, stop=False)

        def emit_sw(acc, g, l, src, np_, dest):
            # S@W contraction kt-MAJOR across both bank slices (first-half
            # S update unlocks 4 of 8 matmuls early), then the pair ACT
            for kt in range(KT):
                for i in range(2):
                    nt = 2 * np_ + i
                    nc.tensor.matmul(
                        acc[:, i, :],
                        w_ap(g, l, kt, nt * 128, (nt + 1) * 128),
                        src[:, kt, :], start=False, stop=(kt == KT - 1))
            nc.scalar.activation(dest[:, 2 * np_:2 * np_ + 2, :], acc[:], Tanh)

        def emit_gate_pair(g, l, xt, src, np_, dest):
            acc = psum.tile([128, 2, BT], FP, name="acc")
            emit_xu(acc, g, l, xt, np_)
            emit_sw(acc, g, l, src, np_, dest)

        def emit_layer(l, xt, s, cover=None):
            rt = rpool.tile([128, NT, BT], BF)
            zt = zpool.tile([128, NT, BT], BF)
            gt = gpool.tile([128, NT, BT], BF)
            ht = hpool.tile([128, NT, BT], BF)
            # boundary: cover work (final of chunk c-1 / S0 of chunk c+2)
            # then the hoisted S-independent X@U matmuls of the first two
            # pair groups -- all pure-PE work that runs while the previous
            # layer's ACT->DVE S-update chain drains.  (cover's S0 group +
            # two hoisted accs = exactly the 3 psum bufs.)
            if cover is not None:
                cover()
            acc_r0 = psum.tile([128, 2, BT], FP, name="acc")
            acc_r1 = psum.tile([128, 2, BT], FP, name="acc")
            emit_xu(acc_r0, "r", l, xt, 0)
            emit_xu(acc_r1, "r", l, xt, 1)
            # R first: hides the R-ACT -> R-mul -> H chain under Z/G work
            emit_sw(acc_r0, "r", l, s, 0, rt)
            emit_sw(acc_r1, "r", l, s, 1, rt)
            emit_gate_pair("z", l, xt, s, 0, zt)
            emit_gate_pair("z", l, xt, s, 1, zt)
            # DVE batch 1: R <- S*R (feeds H); Z <- Z*S (in place, reads
            # the OLD S before the layer-end sub overwrites it)
            for hf in range(2):
                cs = slice(2 * hf, 2 * hf + 2)
                nc.vector.tensor_mul(rt[:, cs, :], s[:, cs, :], rt[:, cs, :])
            for hf in range(2):
                cs = slice(2 * hf, 2 * hf + 2)
                nc.vector.tensor_mul(zt[:, cs, :], zt[:, cs, :], s[:, cs, :])
            for np_ in range(NP):
                emit_gate_pair("g", l, xt, s, np_, gt)
            for np_ in range(NP):
                emit_gate_pair("h", l, xt, rt, np_, ht)
            # DVE batch 2: S = (Z*S) - (G-1)*H, half-gate granular
            for hf in range(2):
                cs = slice(2 * hf, 2 * hf + 2)
                nc.vector.scalar_tensor_tensor(
                    gt[:, cs, :], gt[:, cs, :], 1.0, ht[:, cs, :],
                    op0=sub, op1=mult)          # (G-1)*H
                nc.vector.tensor_sub(s[:, cs, :], zt[:, cs, :], gt[:, cs, :])

        def emit_final(c, s, xt_live):
            # out = S @ Wf + bf (bf lands via a K=1 matmul on the ones row)
            accf = psum_f.tile([1, BT], FP)
            nc.tensor.matmul(accf[:], bfc[:], xt_live[0:1, :],
                             start=True, stop=False)
            for kt in range(KT):
                nc.tensor.matmul(accf[:], wf[:, kt:kt + 1], s[:, kt, :],
                                 start=False, stop=(kt == KT - 1))
            ot = opool.tile([1, BT], FP)
            nc.scalar.activation(ot[:], accf[:],
                                 mybir.ActivationFunctionType.Copy)
            r0 = c * BT
            nc.sync.dma_start(out=OUT[r0:r0 + BT, 0:1].rearrange("b o -> o b"),
                              in_=ot[:])

        def emit_all():
            n_chunks = B // BT
            xts = {0: load_xt(0), 1: load_xt(1)}
            # startup S0 for the first two chunks (batch-halved chunk 0 so
            # the PE starts on the first xt half-transfer)
            s = spool.tile([128, KT, BT], BF, name="s")
            for h in range(2):
                c0, c1 = h * 256, (h + 1) * 256
                for np_ in range(NP):
                    acc = psum.tile([128, 2, BT], FP, name="acc")
                    for i in range(2):
                        nt = 2 * np_ + i
                        nc.tensor.matmul(acc[:, i, 0:256],
                                         w0[:, nt * 128:(nt + 1) * 128],
                                         xts[0][:, c0:c1], start=True,
                                         stop=True)
                    nc.scalar.activation(s[:, 2 * np_:2 * np_ + 2, c0:c1],
                                         acc[:, :, 0:256], Tanh)
            nxt_s = spool.tile([128, KT, BT], BF, name="s")
            for np_ in range(NP):
                emit_s0_pair(xts[1], nxt_s, np_)
            prev = None          # (c, s) whose final is still pending
            nxt2_s = None
            for c in range(n_chunks):
                if c + 2 < n_chunks:
                    xts[c + 2] = load_xt(c + 2)
                xt = xts[c]

                def cover1():
                    # l0->l1 boundary: previous chunk's final (also the last
                    # reader of the spool buffer chunk c+2's S0 overwrites),
                    # then half of chunk c+2's S0
                    nonlocal prev, nxt2_s
                    if prev is not None:
                        emit_final(prev[0], prev[1], xt)
                        prev = None
                    if c + 2 < n_chunks:
                        nxt2_s = spool.tile([128, KT, BT], BF, name="s")
                        emit_s0_pair(xts[c + 2], nxt2_s, 0)

                def cover2():
                    if c + 2 < n_chunks:
                        emit_s0_pair(xts[c + 2], nxt2_s, 1)

                emit_layer(0, xt, s)
                emit_layer(1, xt, s, cover=cover1)
                emit_layer(2, xt, s, cover=cover2)
                prev = (c, s)
                s, nxt_s = nxt_s, nxt2_s
            emit_final(prev[0], prev[1], xts[n_chunks - 1])

        emit_weight_dmas()
        if reps == 1:
            emit_all()
        else:           # device-side repetition loop, for benchmarking only
            with tc.For_i(0, reps):
                emit_all()

    nc.compile()
    return nc


_NC = None


def _get_nc():
    global _NC
    if _NC is None:
        _NC = _build()
    return _NC


def _bf(a):
    return np.ascontiguousarray(
        np.asarray(a, np.float32).astype(ml_dtypes.bfloat16))


def prep_shared(inputs):
    """bf16-convert weights; augment U-type weights with their bias as
    ROW 0 (matches the ones row at partition 0 of XT)."""
    shared = {}
    for g in GATES:
        shared[f"W{g}"] = _bf(inputs[f"W{g}"])
        U = np.asarray(inputs[f"U{g}"], np.float32)          # [L, D, N]
        b = np.asarray(inputs[f"b{g}"], np.float32)          # [L, 1, N]
        shared[f"U{g}a"] = _bf(
            np.concatenate([b.reshape(L, 1, N), U], axis=1))  # [L, DA, N]
    W0 = np.asarray(inputs["W0"], np.float32)                # [D, N]
    b0 = np.asarray(inputs["b0"], np.float32)                # [1, N]
    shared["W0a"] = _bf(np.concatenate([b0, W0], axis=0))
    shared["Wf"] = _bf(inputs["Wf"])
    shared["bfc"] = _bf(np.asarray(inputs["bf"], np.float32).reshape(1, 1))
    return shared


def prep_xt(Xcore):
    """[B, D] batch-major core shard -> [DA, B] bf16 feature-major,
    ones row 0."""
    Xt = np.asarray(Xcore, np.float32).T                     # [D, B]
    ones = np.ones((1, Xt.shape[1]), np.float32)
    return _bf(np.concatenate([ones, Xt], axis=0))


def _run(inputs, **kw):
    nc = _get_nc()
    shared = prep_shared(inputs)
    X = np.asarray(inputs["X"], np.float32)
    in_maps = [dict(shared, XT=prep_xt(X[i * B:(i + 1) * B]))
               for i in range(N_CORES)]
    res = run_bass_kernel_spmd(nc, in_maps, list(range(N_CORES)), **kw)
    out = np.concatenate([res.results[i]["out"] for i in range(N_CORES)], axis=0)
    return out, res


def kernel(**inputs) -> np.ndarray:
    out, _ = _run(inputs)
    return out


# revision 15
# speedup vs baseline: 1.0863x; 1.0062x over previous
"""DGM-net forward kernel for Trainium2, 8-core data parallel.

Network (per batch row x of width 101, n_nodes=512, 3 layers):
    S = tanh(x @ W0 + b0)
    for i in 0..2:
        Z = tanh(x @ Uz[i] + S @ Wz[i] + bz[i])
        G = tanh(x @ Ug[i] + S @ Wg[i] + bg[i])
        R = tanh(x @ Ur[i] + S @ Wr[i] + br[i])
        H = tanh(x @ Uh[i] + (S*R) @ Wh[i] + bh[i])
        S = (1-G)*H + Z*S
    out = S @ Wf + bf

Layout: activations feature-major ([feature partitions, batch free]) so
every matmul uses the weight in NATURAL layout as stationary lhsT and
the activation as moving rhs.  X is host-transposed and shipped as bf16
"XT" with a ones row at partition 0; every bias is folded into the
matmul (U/W0 stationaries carry the bias as row 0), so ACT instructions
are bias-free and can span two PSUM banks.

HW-microbenchmarked facts (mb.py, REAL random data -- the zero-data
regime is ~1.5x faster and misleading):
  bf16  MM [128x128]x[128x512]    ~199 ns   <- fastest real-data option
  fp32r MM                        ~220 ns
  ACT tanh pair [128,2x512]       ~690 ns
  DVE op [128,512]                ~370 ns
  cross-engine dependency edge    ~1.1 us   -- 10x the cost model
The whole compute path is bf16 (numpy-validated rel err 7e-3 vs 2e-2
budget; PSUM accumulation stays fp32): 10% faster matmul stream AND
half the SBUF/DMA.  The ~1.1us/edge handoff makes serial MM->ACT->DVE->
MM chains expensive, so every layer boundary is covered with independent
PE work: the previous chunk's deferred final, chunk c+2's S0, and the
next layer's hoisted X@U matmuls; within a group the S@W contraction is
kt-major so the first-half S update unlocks half the matmuls early.
"""
import numpy as np
import ml_dtypes
from contextlib import ExitStack

import concourse.bacc as bacc
import concourse.mybir as mybir
import concourse.tile as tile
from concourse.bass_utils import run_bass_kernel_spmd


N_CORES = 8
B_FULL = 65536
B = B_FULL // N_CORES      # rows per core
D = 101                    # input width
DA = D + 1                 # augmented with ones row (bias fold)
N = 512                    # n_nodes
L = 3                      # layers
BT = 512                   # batch chunk (free dim of matmuls)
NT = N // 128              # output-feature tiles per gate
KT = N // 128              # contraction tiles for S@W
NP = NT // 2               # two-bank pair groups per gate
FP = mybir.dt.float32
BF = mybir.dt.bfloat16

GATES = ("z", "g", "r", "h")


def _build(reps=1):
    nc = bacc.Bacc(None)
    Tanh = mybir.ActivationFunctionType.Tanh

    XTd = nc.declare_dram_parameter("XT", [DA, B], BF, isOutput=False)
    W0d = nc.declare_dram_parameter("W0a", [DA, N], BF, isOutput=False)
    Ud = {g: nc.declare_dram_parameter(f"U{g}a", [L, DA, N], BF, isOutput=False)
          for g in GATES}
    Wd = {g: nc.declare_dram_parameter(f"W{g}", [L, N, N], BF, isOutput=False)
          for g in GATES}
    Wfd = nc.declare_dram_parameter("Wf", [N, 1], BF, isOutput=False)
    bfd = nc.declare_dram_parameter("bfc", [1, 1], BF, isOutput=False)
    OUT = nc.declare_dram_parameter("out", [B, 1], FP, isOutput=True)

    with tile.TileContext(nc) as tc, ExitStack() as ctx:
        consts = ctx.enter_context(tc.tile_pool(name="consts", bufs=1))
        xtpool = ctx.enter_context(tc.tile_pool(name="xt", bufs=4))
        spool = ctx.enter_context(tc.tile_pool(name="s", bufs=4))
        zpool = ctx.enter_context(tc.tile_pool(name="z", bufs=2))
        gpool = ctx.enter_context(tc.tile_pool(name="g", bufs=2))
        rpool = ctx.enter_context(tc.tile_pool(name="r", bufs=2))
        hpool = ctx.enter_context(tc.tile_pool(name="h", bufs=2))
        opool = ctx.enter_context(tc.tile_pool(name="o", bufs=2))
        # pair-granular PSUM: each tile spans TWO banks ([128, 2, 512] fp32)
        psum = ctx.enter_context(tc.tile_pool(name="psum", bufs=3, space="PSUM"))
        psum_f = ctx.enter_context(tc.tile_pool(name="psum_f", bufs=2, space="PSUM"))

        # --- resident weights, natural (k-major) layout, via SWDGE ---
        def wdma(out, in_):
            nc.gpsimd.dma_start(out=out, in_=in_)

        w0 = consts.tile([DA, N], BF)
        bfc = consts.tile([1, 1], BF)
        u0, w0g, u12, w12 = {}, {}, {}, {}
        for g in GATES:
            u0[g] = consts.tile([DA, N], BF, name=f"u0_{g}")
            w0g[g] = consts.tile([128, KT, N], BF, name=f"w0_{g}")
            u12[g] = consts.tile([DA, L - 1, N], BF, name=f"u12_{g}")
            w12[g] = consts.tile([128, L - 1, KT, N], BF, name=f"w12_{g}")
        wf = consts.tile([128, KT], BF)

        def u_ap(g, l, c0, c1):
            return u0[g][:, c0:c1] if l == 0 else u12[g][:, l - 1, c0:c1]

        def w_ap(g, l, kt, c0, c1):
            return (w0g[g][:, kt, c0:c1] if l == 0
                    else w12[g][:, l - 1, kt, c0:c1])

        def emit_weight_dmas():
            nc.sync.dma_start(out=bfc[:], in_=bfd[:])
            wdma(w0[:], W0d[:])
            # first-consumed layer-0 weights first (gate order r,z,g,h)
            for g in ("r", "z", "g", "h"):
                wdma(u0[g][:], Ud[g][0].rearrange("p n -> p n"))
                wdma(w0g[g][:, 0:2],
                     Wd[g][0, 0:256].rearrange("(kt p) n -> p kt n", p=128))
                wdma(w0g[g][:, 2:4],
                     Wd[g][0, 256:512].rearrange("(kt p) n -> p kt n", p=128))
            for g in ("r", "z", "g", "h"):
                wdma(u12[g][:], Ud[g][1:3].rearrange("l p n -> p l n"))
                wdma(w12[g][:, 0],
                     Wd[g][1].rearrange("(kt p) n -> p kt n", p=128))
            for g in ("r", "z", "g", "h"):
                wdma(w12[g][:, 1],
                     Wd[g][2].rearrange("(kt p) n -> p kt n", p=128))
            wdma(wf[:], Wfd[:].rearrange("(kt p) o -> p (kt o)", p=128))

        sub, mult = mybir.AluOpType.subtract, mybir.AluOpType.mult

        def load_xt(c):
            xt = xtpool.tile([DA, BT], BF)
            if c == 0:
                h = BT // 2
                nc.sync.dma_start(out=xt[:, 0:h], in_=XTd[:, 0:h])
                nc.sync.dma_start(out=xt[:, h:BT], in_=XTd[:, h:BT])
            else:
                eng = nc.scalar if c == 1 else nc.sync
                eng.dma_start(out=xt[:], in_=XTd[:, c * BT:(c + 1) * BT])
            return xt

        def emit_s0_pair(xt, s, np_):
            # S0 = tanh(X_aug @ W0_aug), one two-bank pair group
            acc = psum.tile([128, 2, BT], FP, name="acc")
            for i in range(2):
                nt = 2 * np_ + i
                nc.tensor.matmul(acc[:, i, :], w0[:, nt * 128:(nt + 1) * 128],
                                 xt[:], start=True, stop=True)
            nc.scalar.activation(s[:, 2 * np_:2 * np_ + 2, :], acc[:], Tanh)

        def emit_xu(acc, g, l, xt, np_):
            # the two S-independent X@U matmuls of a pair group
            for i in range(2):
                nt = 2 * np_ + i
                nc.tensor.matmul(
                    acc[:, i, :], u_ap(g, l, nt * 128, (nt + 1) * 128),
                    xt[:], start=True, stop=False)

        def emit_sw(acc, g, l, src, np_, dest):
            # S@W contraction kt-MAJOR across both bank slices (first-half
            # S update unlocks 4 of 8 matmuls early), then the pair ACT
            for kt in range(KT):
                for i in range(2):
                    nt = 2 * np_ + i
                    nc.tensor.matmul(
                        acc[:, i, :],
                        w_ap(g, l, kt, nt * 128, (nt + 1) * 128),
                        src[:, kt, :], start=False, stop=(kt == KT - 1))
            nc.scalar.activation(dest[:, 2 * np_:2 * np_ + 2, :], acc[:], Tanh)

        def emit_gate_pair(g, l, xt, src, np_, dest):
            acc = psum.tile([128, 2, BT], FP, name="acc")
            emit_xu(acc, g, l, xt, np_)
            emit_sw(acc, g, l, src, np_, dest)

        def chunk_units(c, xt, s):
            """Generator: 31 work units (10 per layer + final).  Two of
            these run phase-offset-interleaved so every dependency stall
            of one chunk is covered by the other's matmul units."""
            for l in range(L):
                rt = rpool.tile([128, NT, BT], BF)
                zt = zpool.tile([128, NT, BT], BF)
                gt = gpool.tile([128, NT, BT], BF)
                ht = hpool.tile([128, NT, BT], BF)
                # R first: hides the R-ACT -> R-mul -> H chain under Z/G
                for np_ in range(NP):
                    emit_gate_pair("r", l, xt, s, np_, rt)
                    yield
                for np_ in range(NP):
                    emit_gate_pair("z", l, xt, s, np_, zt)
                    yield
                # DVE 1: R <- S*R (feeds H); Z <- Z*S (in place, reads the
                # OLD S before the layer-end sub overwrites it)
                for hf in range(2):
                    cs = slice(2 * hf, 2 * hf + 2)
                    nc.vector.tensor_mul(rt[:, cs, :], s[:, cs, :],
                                         rt[:, cs, :])
                for hf in range(2):
                    cs = slice(2 * hf, 2 * hf + 2)
                    nc.vector.tensor_mul(zt[:, cs, :], zt[:, cs, :],
                                         s[:, cs, :])
                yield
                for np_ in range(NP):
                    emit_gate_pair("g", l, xt, s, np_, gt)
                    yield
                for np_ in range(NP):
                    emit_gate_pair("h", l, xt, rt, np_, ht)
                    yield
                # DVE 2: S = (Z*S) - (G-1)*H, half-gate granular
                for hf in range(2):
                    cs = slice(2 * hf, 2 * hf + 2)
                    nc.vector.scalar_tensor_tensor(
                        gt[:, cs, :], gt[:, cs, :], 1.0, ht[:, cs, :],
                        op0=sub, op1=mult)          # (G-1)*H
                    nc.vector.tensor_sub(s[:, cs, :], zt[:, cs, :],
                                         gt[:, cs, :])
                yield
            emit_final(c, s, xt)
            yield

        def emit_final(c, s, xt_live):
            # out = S @ Wf + bf (bf lands via a K=1 matmul on the ones row)
            accf = psum_f.tile([1, BT], FP)
            nc.tensor.matmul(accf[:], bfc[:], xt_live[0:1, :],
                             start=True, stop=False)
            for kt in range(KT):
                nc.tensor.matmul(accf[:], wf[:, kt:kt + 1], s[:, kt, :],
                                 start=False, stop=(kt == KT - 1))
            ot = opool.tile([1, BT], FP)
            nc.scalar.activation(ot[:], accf[:],
                                 mybir.ActivationFunctionType.Copy)
            r0 = c * BT
            nc.sync.dma_start(out=OUT[r0:r0 + BT, 0:1].rearrange("b o -> o b"),
                              in_=ot[:])

        # slot pattern per 20-slot period: strict alternation EXCEPT a
        # double-pull of the partner right after each chunk's layer-end
        # DVE unit, so the dependent next-layer matmuls always have ~4us
        # of foreign PE work queued ahead of them in the PE FIFO.
        PATTERN = "ABABABABABBABABABABA"
        SENTINEL = object()

        def pull(g):
            return next(g, SENTINEL) is not SENTINEL

        def emit_all():
            n_chunks = B // BT
            xts = {0: load_xt(0), 1: load_xt(1)}
            # startup S0 for the first pair (batch-halved chunk 0 so the
            # PE starts on the first xt half-transfer)
            s_a = spool.tile([128, KT, BT], BF, name="s")
            for h in range(2):
                c0, c1 = h * 256, (h + 1) * 256
                for np_ in range(NP):
                    acc = psum.tile([128, 2, BT], FP, name="acc")
                    for i in range(2):
                        nt = 2 * np_ + i
                        nc.tensor.matmul(acc[:, i, 0:256],
                                         w0[:, nt * 128:(nt + 1) * 128],
                                         xts[0][:, c0:c1], start=True,
                                         stop=True)
                    nc.scalar.activation(s_a[:, 2 * np_:2 * np_ + 2, c0:c1],
                                         acc[:, :, 0:256], Tanh)
            s_b = spool.tile([128, KT, BT], BF, name="s")
            for np_ in range(NP):
                emit_s0_pair(xts[1], s_b, np_)
            for p in range(n_chunks // 2):
                ca, cb = 2 * p, 2 * p + 1
                if ca + 2 < n_chunks:
                    xts[ca + 2] = load_xt(ca + 2)
                if cb + 2 < n_chunks:
                    xts[cb + 2] = load_xt(cb + 2)
                ga = chunk_units(ca, xts[ca], s_a)
                gb = chunk_units(cb, xts[cb], s_b)
                for _ in range(5):          # phase offset: A runs 5 ahead
                    pull(ga)
                a_alive = b_alive = True
                while a_alive:
                    for ch in PATTERN:
                        if ch == "A":
                            a_alive = pull(ga) and a_alive
                        else:
                            b_alive = pull(gb) and b_alive
                # drain B, interleaving the next pair's S0 pair-groups as
                # cover for B's tail chains (next-A's S0 first: the next
                # pair's head consumes s_a2 almost immediately)
                tails = []
                if ca + 2 < n_chunks:
                    s_a2 = spool.tile([128, KT, BT], BF, name="s")
                    s_b2 = spool.tile([128, KT, BT], BF, name="s")
                    tails = [lambda np_=np_: emit_s0_pair(xts[ca + 2], s_a2, np_)
                             for np_ in range(NP)]
                    tails += [lambda np_=np_: emit_s0_pair(xts[cb + 2], s_b2, np_)
                              for np_ in range(NP)]
                    s_a, s_b = s_a2, s_b2
                while b_alive:
                    b_alive = pull(gb)
                    if tails:
                        tails.pop(0)()
                while tails:
                    tails.pop(0)()

        emit_weight_dmas()
        if reps == 1:
            emit_all()
        else:           # device-side repetition loop, for benchmarking only
            with tc.For_i(0, reps):
                emit_all()

    nc.compile()
    return nc


_NC = None


def _get_nc():
    global _NC
    if _NC is None:
        _NC = _build()
    return _NC


def _bf(a):
    return np.ascontiguousarray(
        np.asarray(a, np.float32).astype(ml_dtypes.bfloat16))


def prep_shared(inputs):
    """bf16-convert weights; augment U-type weights with their bias as
    ROW 0 (matches the ones row at partition 0 of XT)."""
    shared = {}
    for g in GATES:
        shared[f"W{g}"] = _bf(inputs[f"W{g}"])
        U = np.asarray(inputs[f"U{g}"], np.float32)          # [L, D, N]
        b = np.asarray(inputs[f"b{g}"], np.float32)          # [L, 1, N]
        shared[f"U{g}a"] = _bf(
            np.concatenate([b.reshape(L, 1, N), U], axis=1))  # [L, DA, N]
    W0 = np.asarray(inputs["W0"], np.float32)                # [D, N]
    b0 = np.asarray(inputs["b0"], np.float32)                # [1, N]
    shared["W0a"] = _bf(np.concatenate([b0, W0], axis=0))
    shared["Wf"] = _bf(inputs["Wf"])
    shared["bfc"] = _bf(np.asarray(inputs["bf"], np.float32).reshape(1, 1))
    return shared


def prep_xt(Xcore):
    """[B, D] batch-major core shard -> [DA, B] bf16 feature-major,
    ones row 0."""
    Xt = np.asarray(Xcore, np.float32).T                     # [D, B]
    ones = np.ones((1, Xt.shape[1]), np.float32)
    return _bf(np.concatenate([ones, Xt], axis=0))


def _run(inputs, **kw):
    nc = _get_nc()
    shared = prep_shared(inputs)
    X = np.asarray(inputs["X"], np.float32)
    in_maps = [dict(shared, XT=prep_xt(X[i * B:(i + 1) * B]))
               for i in range(N_CORES)]
    res = run_bass_kernel_spmd(nc, in_maps, list(range(N_CORES)), **kw)
    out = np.concatenate([res.results[i]["out"] for i in range(N_CORES)], axis=0)
    return out, res


def kernel(**inputs) -> np.ndarray:
    out, _ = _run(inputs)
    return out
